# revision 11
# baseline (speedup 1.0000x reference)
"""Tensor-parallel causal attention block (qkv proj + RoPE + attention + out proj)
for Trainium2, sharded over 8 NeuronCores by attention head (2 heads/core).

Contract: kernel(**inputs) takes the FULL inputs (x [1,2048,1024] f32,
w_in [3072,1024] f32, w_out [1024,1024] f32, is_causal scalar) and returns the
FULL output [1,2048,1024] f32.

Per-core layout strategy (everything kept transposed, [feature, seq], so no
on-device transposes of activations are ever needed):
  - host pre-transposes x -> xT [1024,2048] and the weight shards
  - qkvT = w_shard @ xT  ->  [384, 2048] (Q.T | K.T | V.T rows, 2 heads packed)
  - RoPE applied in [hd, s] layout via a constant rotation matmul + elementwise
  - scores computed transposed: S.T[k, q] = K @ Q.T  (softmax dim = partitions)
  - probs (unnormalized exp) hit PV directly:  ctx.T = V_aug.T @ P.T, where
    V_aug carries a ones column so row 64 of the PV output is the softmax
    denominator; normalization happens after PV via reciprocal + PE broadcast
  - out partial = ctx @ w_out_shard.T; host sums the 8 partials (the TP
    all-reduce is a plain numpy sum of disjoint-head partials).
All matmuls run in float32r (full-rate fp32 on the PE, ~1e-4 relative).
"""
import sys

sys.path.insert(0, '/opt/trn_rl_repo')

from contextlib import ExitStack

import numpy as np

import concourse.bass as bass
from concourse import mybir, tile
from concourse.bass_utils import run_bass_kernel_spmd

B, S, D, H = 1, 2048, 1024, 16
HD = D // H            # 64
NCORES = 8
HPC = H // NCORES      # heads per core = 2
EPC = HPC * HD         # features per core = 128
ROPE_BASE = 10000.0

F32 = mybir.dt.float32
F32R = mybir.dt.float32r

QC = 512               # q-chunk width (one PSUM bank of fp32)
NQC = S // QC          # 4 q-chunks
NST = S // 128         # 16 s-tiles / k-tiles
ND = D // 128          # 8 contraction tiles for the input projection


def _split_multi_waits(nc, max_waits=1):
    """This container's walrus build accepts at most one embedded sync wait per
    instruction; move extra waits onto preceding same-engine NoOps."""
    n_split = 0
    for fn in nc.m.functions:
        for blk in fn.blocks:
            new_insts = []
            for inst in blk.instructions:
                si = inst.sync_info
                waits = list(si.on_wait) if (si and si.on_wait) else []
                if len(waits) > max_waits and inst.engine is not None:
                    for w in waits[max_waits:]:
                        nop = mybir.InstNoOp(
                            name=f"{inst.name}_wn{n_split}", ins=[], outs=[])
                        n_split += 1
                        nop.engine = inst.engine
                        nop.sync_info = mybir.SyncInfo(on_wait=[w], on_update=[])
                        nc.register_instruction(nop, overwrite=True)
                        new_insts.append(nop)
                    si.on_wait = waits[:max_waits]
                new_insts.append(inst)
            blk.instructions[:] = new_insts
    return n_split


def _host_constants():
    inv_freq = 1.0 / (ROPE_BASE ** (np.arange(0, HD, 2, dtype=np.float64) / HD))
    t = np.arange(S, dtype=np.float64)
    freqs = np.outer(inv_freq, t)                    # [32, S]  ([hd, s] layout)
    emb = np.concatenate([freqs, freqs], axis=0)     # [64, S]
    cosT = np.cos(emb).astype(np.float32)
    sinT = np.sin(emb).astype(np.float32)
    cos2 = np.tile(cosT, (2, 1))                     # [128, S] (2 heads packed)
    sin2 = np.tile(sinT, (2, 1))
    # rotate_half as a matrix: (R q)[i] = -q[i+32] (i<32), q[i-32] (i>=32)
    R = np.zeros((HD, HD), dtype=np.float32)
    for i in range(HD // 2):
        R[i, i + HD // 2] = -1.0
        R[i + HD // 2, i] = 1.0
    R2 = np.zeros((128, 128), dtype=np.float32)
    R2[0:64, 0:64] = R
    R2[64:128, 64:128] = R
    rotT = np.ascontiguousarray(R2.T)
    # upper-triangular (k<=q) mask for the diagonal 128x128 blocks of S.T[k,q]
    tri = np.triu(np.ones((128, 128), dtype=np.float32))
    ident = np.eye(128, dtype=np.float32)
    return cos2, sin2, rotT, tri, ident


def _build_program(causal: bool):
    nc = bass.Bass()
    xT_d = nc.dram_tensor("xT", [D, S], F32R, kind="ExternalInput")
    winT_d = nc.dram_tensor("winT", [D, 3 * EPC], F32R, kind="ExternalInput")
    woT0_d = nc.dram_tensor("woT0", [HD, D], F32R, kind="ExternalInput")
    woT1_d = nc.dram_tensor("woT1", [HD, D], F32R, kind="ExternalInput")
    pout_d = nc.dram_tensor("pout", [S, D], F32, kind="ExternalOutput")

    cos2_np, sin2_np, rotT_np, tri_np, ident_np = _host_constants()
    cos2_d = nc.inline_tensor(cos2_np, name="cos2")
    sin2_d = nc.inline_tensor(sin2_np, name="sin2")
    rotT_d = nc.dram_tensor("rotT", [128, 128], F32R, kind="ExternalInput")
    tri_d = nc.dram_tensor("tri", [128, 128], F32R, kind="ExternalInput")
    ident_d = nc.dram_tensor("ident", [128, 128], F32R, kind="ExternalInput")

    with tile.TileContext(nc) as tc, ExitStack() as ctx:
        sb = ctx.enter_context(tc.tile_pool(name="sb", bufs=1))

        # ---- persistent SBUF tensors -----------------------------------
        winT = sb.tile([128, ND * 3 * EPC], F32R, name="winT")      # 12KB/p
        for d in range(ND):
            nc.sync.dma_start(winT[:, d * 3 * EPC:(d + 1) * 3 * EPC],
                              winT_d[d * 128:(d + 1) * 128, :])
        woT0 = sb.tile([HD, D], F32R, name="woT0")
        nc.sync.dma_start(woT0[:], woT0_d[:, :])
        woT1 = sb.tile([HD, D], F32R, name="woT1")
        nc.sync.dma_start(woT1[:], woT1_d[:, :])
        cos2 = sb.tile([128, S], F32, name="cos2")
        nc.sync.dma_start(cos2[:], cos2_d[:, :])
        sin2 = sb.tile([128, S], F32, name="sin2")
        nc.sync.dma_start(sin2[:], sin2_d[:, :])
        rot = sb.tile([128, 128], F32R, name="rot")
        nc.sync.dma_start(rot[:], rotT_d[:, :])
        tri = sb.tile([128, 128], F32R, name="tri")
        nc.sync.dma_start(tri[:], tri_d[:, :])
        ident = sb.tile([128, 128], F32R, name="ident")
        nc.sync.dma_start(ident[:], ident_d[:, :])
        onesf = sb.tile([128, HD], F32, name="onesf")
        nc.vector.memset(onesf[:], 1.0)

        qraw = sb.tile([128, S], F32R, name="qraw")
        kraw = sb.tile([128, S], F32R, name="kraw")
        vtr = sb.tile([128, S], F32R, name="vtr")      # V.T [e, s]
        qrot = sb.tile([128, S], F32R, name="qrot")    # Q.T after RoPE
        krot = sb.tile([128, S], F32R, name="krot")
        # V natural + ones cols: per s-tile 130 cols: [V_h0(64) 1 V_h1(64) 1]
        vnat = sb.tile([128, NST * 130], F32R, name="vnat")
        nc.vector.memset(vnat[:].bitcast(F32), 1.0)
        heatout = sb.tile([1, 1], F32, name="heatout")
        ctx0 = sb.tile([HD, S], F32R, name="ctx0")     # ctx.T head0 (base 0)
        ctx1 = sb.tile([HD, S], F32R, name="ctx1")     # ctx.T head1 (base 0)

        # ================= Stage A: QKV.T projection ====================
        with tc.tile_pool(name="psA", bufs=1, space="PSUM") as psA, \
             tc.tile_pool(name="wka", bufs=3) as wka:
            for h in range(2):                     # s-halves (PSUM capacity)
                s0 = h * (S // 2)
                accs = []
                for et in range(3):
                    for sch in range(2):
                        a = psA.tile([128, QC], F32, tag="acc", bufs=6, name="acc")
                        accs.append(a)
                for d in range(ND):
                    xt = wka.tile([128, S // 2], F32R, tag="xt", name="xt")
                    nc.sync.dma_start(
                        xt[:], xT_d[d * 128:(d + 1) * 128, s0:s0 + S // 2])
                    for et in range(3):
                        lw = winT[:, d * 3 * EPC + et * 128:
                                  d * 3 * EPC + (et + 1) * 128]
                        for sch in range(2):
                            nc.tensor.matmul(
                                accs[et * 2 + sch][:], lw,
                                xt[:, sch * QC:(sch + 1) * QC],
                                start=(d == 0), stop=(d == ND - 1))
                # drain Q/K/V.T to SBUF
                for sch in range(2):
                    c0 = s0 + sch * QC
                    nc.scalar.copy(qraw[:, c0:c0 + QC], accs[0 * 2 + sch][:])
                    nc.scalar.copy(kraw[:, c0:c0 + QC], accs[1 * 2 + sch][:])
                    nc.scalar.copy(vtr[:, c0:c0 + QC], accs[2 * 2 + sch][:])
                # RoPE for this half (overlaps next half's projection mms)
                for (raw, out) in ((qraw, qrot), (kraw, krot)):
                    for sch in range(2):
                        c0 = s0 + sch * QC
                        rp = psA.tile([128, QC], F32, tag="acc", bufs=6,
                                      name="rp")
                        nc.tensor.matmul(rp[:], rot[:], raw[:, c0:c0 + QC],
                                         start=True, stop=True)
                        t1 = wka.tile([128, QC], F32R, tag="t1", name="t1")
                        nc.vector.tensor_mul(t1[:], raw[:, c0:c0 + QC],
                                             cos2[:, c0:c0 + QC])
                        t2 = wka.tile([128, QC], F32R, tag="t2", name="t2")
                        nc.vector.tensor_mul(t2[:], rp[:], sin2[:, c0:c0 + QC])
                        nc.vector.tensor_add(out[:, c0:c0 + QC], t1[:], t2[:])
                # V natural via PE transpose for this half
                for j in range(h * 8, h * 8 + 8):
                    vp = psA.tile([128, 128], F32R, tag="vt", bufs=2,
                                  name="vp")
                    nc.tensor.transpose(vp[:], vtr[:, j * 128:(j + 1) * 128],
                                        ident[:])
                    nc.vector.tensor_copy(vnat[:, j * 130:j * 130 + 64],
                                          vp[:, 0:64])
                    nc.vector.tensor_copy(vnat[:, j * 130 + 65:j * 130 + 129],
                                          vp[:, 64:128])

        # ============ Stage B: attention + Stage C: out proj ============
        with tc.tile_pool(name="psB", bufs=1, space="PSUM") as psB, \
             tc.tile_pool(name="wkb", bufs=3) as wkb:

            def emit_norm_and_outproj(qc, pvs):
                q0 = qc * QC
                for hh in range(2):
                    pv = pvs[hh]
                    rcp = wkb.tile([65, QC], F32, tag="rcp", bufs=2,
                                   name="rcp")
                    with nc.allow_low_precision(reason="denominator recip"):
                        nc.vector.reciprocal(rcp[64:65, :], pv[64:65, :])
                    rb = psB.tile([HD, QC], F32, tag="op", bufs=2, name="rb")
                    nc.tensor.matmul(rb[:], onesf[64:65, 0:HD],
                                     rcp[64:65, :], start=True, stop=True)
                    rbs = wkb.tile([HD, QC], F32, tag="rbs", bufs=2,
                                   name="rbs")
                    nc.scalar.copy(rbs[:], rb[:])
                    ctxh = ctx0 if hh == 0 else ctx1
                    nc.vector.tensor_mul(ctxh[:, q0:q0 + QC], pv[0:64, :],
                                         rbs[:])
                for sti in range(4):
                    c0 = (qc * 4 + sti) * 128
                    ob = wkb.tile([128, D], F32, tag="ob", bufs=3, name="ob")
                    for dc in range(2):
                        op = psB.tile([128, QC], F32, tag="op", bufs=2,
                                      name="op")
                        nc.tensor.matmul(op[:], ctx0[:, c0:c0 + 128],
                                         woT0[:, dc * QC:(dc + 1) * QC],
                                         start=True, stop=False)
                        nc.tensor.matmul(op[:], ctx1[:, c0:c0 + 128],
                                         woT1[:, dc * QC:(dc + 1) * QC],
                                         start=False, stop=True)
                        if dc == 0:
                            nc.scalar.copy(ob[:, dc * QC:(dc + 1) * QC], op[:])
                        else:
                            nc.vector.tensor_copy(
                                ob[:, dc * QC:(dc + 1) * QC], op[:])
                    nc.sync.dma_start(pout_d[c0:c0 + 128, :], ob[:])

            def norm_thunks(qc, pvs):
                """Deferred normalization + out-projection for chunk qc.
                The reciprocals are issued immediately (DVE-only, ~3.4us) so
                the deferred PE work never stalls on them."""
                thunks = []
                rcps = []
                for hh in range(2):
                    rcp = wkb.tile([65, QC], F32, tag="rcp", bufs=4,
                                   name="rcp")
                    with nc.allow_low_precision(reason="denom recip"):
                        nc.vector.reciprocal(rcp[64:65, :], pvs[hh][64:65, :])
                    rcps.append(rcp)

                def norm(hh):
                    def f():
                        q0 = qc * QC
                        pv = pvs[hh]
                        rb = psB.tile([HD, QC], F32, tag="op", bufs=2,
                                      name="rb")
                        nc.tensor.matmul(rb[:], onesf[64:65, 0:HD],
                                         rcps[hh][64:65, :],
                                         start=True, stop=True)
                        rbs = wkb.tile([HD, QC], F32, tag="rbs", bufs=2,
                                       name="rbs")
                        nc.scalar.copy(rbs[:], rb[:])
                        ctxh = ctx0 if hh == 0 else ctx1
                        nc.vector.tensor_mul(ctxh[:, q0:q0 + QC], pv[0:64, :],
                                             rbs[:])
                    return f

                def oproj(sti):
                    def f():
                        c0 = (qc * 4 + sti) * 128
                        ob = wkb.tile([128, D], F32, tag="ob", bufs=3,
                                      name="ob")
                        for dc in range(2):
                            op = psB.tile([128, QC], F32, tag="op", bufs=2,
                                          name="op")
                            nc.tensor.matmul(op[:], ctx0[:, c0:c0 + 128],
                                             woT0[:, dc * QC:(dc + 1) * QC],
                                             start=True, stop=False)
                            nc.tensor.matmul(op[:], ctx1[:, c0:c0 + 128],
                                             woT1[:, dc * QC:(dc + 1) * QC],
                                             start=False, stop=True)
                            if dc == 0:
                                nc.scalar.copy(ob[:, dc * QC:(dc + 1) * QC],
                                               op[:])
                            else:
                                nc.vector.tensor_copy(
                                    ob[:, dc * QC:(dc + 1) * QC], op[:])
                        nc.sync.dma_start(pout_d[c0:c0 + 128, :], ob[:])
                    return f

                thunks.append(norm(0))
                thunks.append(norm(1))
                for sti in range(4):
                    thunks.append(oproj(sti))
                return thunks

            # dependency-free warm-up burst: bridges the RoPE->attention
            # transition gap and re-warms the PE clock (HAM) before the
            # attention stream starts.
            heat = psB.tile([128, QC], F32, tag="st", bufs=2, name="heat")
            for _ in range(36):
                nc.tensor.matmul(heat[:], winT[:, 0:128], winT[:, 1024:1536],
                                 start=True, stop=True)
            nc.scalar.copy(heatout[:], heat[0:1, 0:1])

            LAG = 2
            deferred = []
            for qc in range(NQC):
                q0 = qc * QC
                n_k = 4 * (qc + 1) if causal else NST
                pvs = [psB.tile([65, QC], F32, tag="pv", bufs=4,
                                name=f"pv{hh}") for hh in range(2)]
                window = []

                def emit_pv(pkt, p0, p1, last):
                    js = max(0, pkt - qc * 4) * 128 if causal else 0
                    for hh, pp in ((0, p0), (1, p1)):
                        nc.tensor.matmul(
                            pvs[hh][:, js:QC],
                            vnat[:, pkt * 130 + hh * 65:
                                 pkt * 130 + hh * 65 + 65],
                            pp[:, js:QC], start=(pkt == 0), stop=last)

                for kt in range(n_k):
                    pts = []
                    for hh in range(2):
                        st = psB.tile([128, QC], F32, tag="st", bufs=2,
                                      name="st")
                        nc.tensor.matmul(
                            st[:],
                            krot[hh * 64:(hh + 1) * 64,
                                 kt * 128:(kt + 1) * 128],
                            qrot[hh * 64:(hh + 1) * 64, q0:q0 + QC],
                            start=True, stop=True)
                        pt = wkb.tile([128, QC], F32R, tag="pt", bufs=6,
                                      name="pt")
                        j = kt - qc * 4
                        if causal and j >= 0:
                            nc.scalar.activation(
                                pt[:, j * 128:QC], st[:, j * 128:QC],
                                mybir.ActivationFunctionType.Exp, scale=0.125)
                            nc.vector.tensor_mul(
                                pt[:, j * 128:(j + 1) * 128],
                                pt[:, j * 128:(j + 1) * 128], tri[:])
                        else:
                            nc.scalar.activation(
                                pt[:], st[:],
                                mybir.ActivationFunctionType.Exp, scale=0.125)
                        pts.append(pt)
                    window.append((kt, pts[0], pts[1]))
                    if len(window) > LAG:
                        emit_pv(*window.pop(0), last=False)
                    # interleave one deferred thunk from the previous chunk
                    if deferred and kt % 2 == 1:
                        deferred.pop(0)()
                while window:
                    kt_, a_, b_ = window.pop(0)
                    emit_pv(kt_, a_, b_, last=(kt_ == n_k - 1))
                while deferred:
                    deferred.pop(0)()
                deferred = norm_thunks(qc, pvs)
            while deferred:
                deferred.pop(0)()
    _split_multi_waits(nc)
    return nc


_CONSTS = _host_constants()
_PROGRAMS = {}


def _get_program(causal: bool):
    if causal not in _PROGRAMS:
        _PROGRAMS[causal] = _build_program(causal)
    return _PROGRAMS[causal]


def kernel(x, w_in, w_out, is_causal):
    causal = bool(np.asarray(is_causal).item())
    nc = _get_program(causal)

    x2 = np.asarray(x, dtype=np.float32).reshape(S, D)
    xT = np.ascontiguousarray(x2.T)                       # [D, S]
    w_in = np.asarray(w_in, dtype=np.float32)
    w_out = np.asarray(w_out, dtype=np.float32)

    in_maps = []
    for c in range(NCORES):
        r0 = c * EPC
        wq = w_in[r0:r0 + EPC, :]                          # [128, D]
        wk = w_in[D + r0:D + r0 + EPC, :]
        wv = w_in[2 * D + r0:2 * D + r0 + EPC, :]
        winT = np.ascontiguousarray(
            np.concatenate([wq, wk, wv], axis=0).T)        # [D, 384]
        woT0 = np.ascontiguousarray(w_out[:, r0:r0 + HD].T)        # [64, D]
        woT1 = np.ascontiguousarray(w_out[:, r0 + HD:r0 + EPC].T)  # [64, D]
        in_maps.append({"xT": xT, "winT": winT, "woT0": woT0, "woT1": woT1,
                        "rotT": _CONSTS[2], "tri": _CONSTS[3],
                        "ident": _CONSTS[4]})

    res = run_bass_kernel_spmd(nc, in_maps, list(range(NCORES)))
    out = np.zeros((S, D), dtype=np.float64)
    for c in range(NCORES):
        out += res.results[c]["pout"].astype(np.float64)
    return out.astype(np.float32).reshape(B, S, D)


# revision 13
# speedup vs baseline: 1.0168x; 1.0168x over previous
"""Tensor-parallel causal attention block (qkv proj + RoPE + attention + out proj)
for Trainium2, sharded over 8 NeuronCores by attention head (2 heads/core).

Contract: kernel(**inputs) takes the FULL inputs (x [1,2048,1024] f32,
w_in [3072,1024] f32, w_out [1024,1024] f32, is_causal scalar) and returns the
FULL output [1,2048,1024] f32.

Per-core layout strategy (everything kept transposed, [feature, seq], so no
on-device transposes of activations are ever needed):
  - host pre-transposes x -> xT [1024,2048] and the weight shards
  - qkvT = w_shard @ xT  ->  [384, 2048] (Q.T | K.T | V.T rows, 2 heads packed)
  - RoPE applied in [hd, s] layout via a constant rotation matmul + elementwise
  - scores computed transposed: S.T[k, q] = K @ Q.T  (softmax dim = partitions)
  - probs (unnormalized exp) hit PV directly:  ctx.T = V_aug.T @ P.T, where
    V_aug carries a ones column so row 64 of the PV output is the softmax
    denominator; normalization happens after PV via reciprocal + PE broadcast
  - out partial = ctx @ w_out_shard.T; host sums the 8 partials (the TP
    all-reduce is a plain numpy sum of disjoint-head partials).
All matmuls run in float32r (full-rate fp32 on the PE, ~1e-4 relative).
"""
import sys

sys.path.insert(0, '/opt/trn_rl_repo')

from contextlib import ExitStack

import numpy as np

import concourse.bass as bass
from concourse import mybir, tile
from concourse.bass_utils import run_bass_kernel_spmd

B, S, D, H = 1, 2048, 1024, 16
HD = D // H            # 64
NCORES = 8
HPC = H // NCORES      # heads per core = 2
EPC = HPC * HD         # features per core = 128
ROPE_BASE = 10000.0

F32 = mybir.dt.float32
F32R = mybir.dt.float32r

QC = 512               # q-chunk width (one PSUM bank of fp32)
NQC = S // QC          # 4 q-chunks
NST = S // 128         # 16 s-tiles / k-tiles
ND = D // 128          # 8 contraction tiles for the input projection


def _split_multi_waits(nc, max_waits=1):
    """This container's walrus build accepts at most one embedded sync wait per
    instruction; move extra waits onto preceding same-engine NoOps."""
    n_split = 0
    for fn in nc.m.functions:
        for blk in fn.blocks:
            new_insts = []
            for inst in blk.instructions:
                si = inst.sync_info
                waits = list(si.on_wait) if (si and si.on_wait) else []
                if len(waits) > max_waits and inst.engine is not None:
                    for w in waits[max_waits:]:
                        nop = mybir.InstNoOp(
                            name=f"{inst.name}_wn{n_split}", ins=[], outs=[])
                        n_split += 1
                        nop.engine = inst.engine
                        nop.sync_info = mybir.SyncInfo(on_wait=[w], on_update=[])
                        nc.register_instruction(nop, overwrite=True)
                        new_insts.append(nop)
                    si.on_wait = waits[:max_waits]
                new_insts.append(inst)
            blk.instructions[:] = new_insts
    return n_split


def _host_constants():
    inv_freq = 1.0 / (ROPE_BASE ** (np.arange(0, HD, 2, dtype=np.float64) / HD))
    t = np.arange(S, dtype=np.float64)
    freqs = np.outer(inv_freq, t)                    # [32, S]  ([hd, s] layout)
    emb = np.concatenate([freqs, freqs], axis=0)     # [64, S]
    cosT = np.cos(emb).astype(np.float32)
    sinT = np.sin(emb).astype(np.float32)
    cos2 = np.tile(cosT, (2, 1))                     # [128, S] (2 heads packed)
    sin2 = np.tile(sinT, (2, 1))
    # rotate_half as a matrix: (R q)[i] = -q[i+32] (i<32), q[i-32] (i>=32)
    R = np.zeros((HD, HD), dtype=np.float32)
    for i in range(HD // 2):
        R[i, i + HD // 2] = -1.0
        R[i + HD // 2, i] = 1.0
    R2 = np.zeros((128, 128), dtype=np.float32)
    R2[0:64, 0:64] = R
    R2[64:128, 64:128] = R
    rotT = np.ascontiguousarray(R2.T)
    # upper-triangular (k<=q) mask for the diagonal 128x128 blocks of S.T[k,q]
    tri = np.triu(np.ones((128, 128), dtype=np.float32))
    ident = np.eye(128, dtype=np.float32)
    return cos2, sin2, rotT, tri, ident


def _build_program(causal: bool):
    nc = bass.Bass()
    xT_d = nc.dram_tensor("xT", [D, S], F32R, kind="ExternalInput")
    winT_d = nc.dram_tensor("winT", [D, 3 * EPC], F32R, kind="ExternalInput")
    woT0_d = nc.dram_tensor("woT0", [HD, D], F32R, kind="ExternalInput")
    woT1_d = nc.dram_tensor("woT1", [HD, D], F32R, kind="ExternalInput")
    pout_d = nc.dram_tensor("pout", [S, D], F32, kind="ExternalOutput")

    cos2_np, sin2_np, rotT_np, tri_np, ident_np = _host_constants()
    cos2_d = nc.inline_tensor(cos2_np, name="cos2")
    sin2_d = nc.inline_tensor(sin2_np, name="sin2")
    rotT_d = nc.dram_tensor("rotT", [128, 128], F32R, kind="ExternalInput")
    tri_d = nc.dram_tensor("tri", [128, 128], F32R, kind="ExternalInput")
    ident_d = nc.dram_tensor("ident", [128, 128], F32R, kind="ExternalInput")

    with tile.TileContext(nc) as tc, ExitStack() as ctx:
        sb = ctx.enter_context(tc.tile_pool(name="sb", bufs=1))

        # ---- persistent SBUF tensors -----------------------------------
        winT = sb.tile([128, ND * 3 * EPC], F32R, name="winT")
        for d in range(ND):
            nc.sync.dma_start(winT[:, d * 3 * EPC:(d + 1) * 3 * EPC],
                              winT_d[d * 128:(d + 1) * 128, :])
        woT0 = sb.tile([HD, D], F32R, name="woT0")
        nc.sync.dma_start(woT0[:], woT0_d[:, :])
        woT1 = sb.tile([HD, D], F32R, name="woT1")
        nc.sync.dma_start(woT1[:], woT1_d[:, :])
        cos2 = sb.tile([128, S], F32, name="cos2")
        nc.sync.dma_start(cos2[:], cos2_d[:, :])
        sin2 = sb.tile([128, S], F32, name="sin2")
        nc.sync.dma_start(sin2[:], sin2_d[:, :])
        rot = sb.tile([128, 128], F32R, name="rot")
        nc.sync.dma_start(rot[:], rotT_d[:, :])
        tri = sb.tile([128, 128], F32R, name="tri")
        nc.sync.dma_start(tri[:], tri_d[:, :])
        ident = sb.tile([128, 128], F32R, name="ident")
        nc.sync.dma_start(ident[:], ident_d[:, :])
        onesf = sb.tile([128, HD], F32, name="onesf")
        nc.vector.memset(onesf[:], 1.0)

        qraw = sb.tile([128, S], F32R, name="qraw")
        kraw = sb.tile([128, S], F32R, name="kraw")
        vtr = sb.tile([128, S], F32R, name="vtr")
        qrot = sb.tile([128, S], F32R, name="qrot")
        krot = sb.tile([128, S], F32R, name="krot")
        vnat = sb.tile([128, NST * 130], F32R, name="vnat")
        nc.vector.memset(vnat[:].bitcast(F32), 1.0)
        heatout = sb.tile([1, 1], F32, name="heatout")
        ctx0 = sb.tile([HD, S], F32R, name="ctx0")
        ctx1 = sb.tile([HD, S], F32R, name="ctx1")

        def rope_chunk(c0, psum_tile_fn):
            """RoPE for columns [c0, c0+QC) of q/k (rotation mm + combine)."""
            for (raw, out) in ((qraw, qrot), (kraw, krot)):
                rp = psum_tile_fn()
                nc.tensor.matmul(rp[:], rot[:], raw[:, c0:c0 + QC],
                                 start=True, stop=True)
                t1 = wk0.tile([128, QC], F32R, tag="t1", bufs=3, name="t1")
                nc.vector.tensor_mul(t1[:], raw[:, c0:c0 + QC],
                                     cos2[:, c0:c0 + QC])
                t2 = wk0.tile([128, QC], F32R, tag="t2", bufs=3, name="t2")
                nc.vector.tensor_mul(t2[:], rp[:], sin2[:, c0:c0 + QC])
                nc.vector.tensor_add(out[:, c0:c0 + QC], t1[:], t2[:])

        def vt_tile(j, vt_tile_fn):
            """V natural for s-tile j via PE transpose of V.T."""
            vp = vt_tile_fn()
            nc.tensor.transpose(vp[:, 0:128], vtr[:, j * 128:(j + 1) * 128],
                                ident[:])
            nc.vector.tensor_copy(vnat[:, j * 130:j * 130 + 64], vp[:, 0:64])
            nc.vector.tensor_copy(vnat[:, j * 130 + 65:j * 130 + 129],
                                  vp[:, 64:128])

        wk0 = ctx.enter_context(tc.tile_pool(name="wk0", bufs=1))

        # ========== Stage A, first half (s in [0, 1024)) ================
        with tc.tile_pool(name="psA", bufs=1, space="PSUM") as psA, \
             tc.tile_pool(name="wka", bufs=3) as wka:
            accs = [psA.tile([128, QC], F32, tag="acc", bufs=6, name="acc")
                    for _ in range(6)]
            for d in range(ND):
                xt = wka.tile([128, S // 2], F32R, tag="xt", name="xt")
                nc.sync.dma_start(xt[:], xT_d[d * 128:(d + 1) * 128, 0:S // 2])
                for et in range(3):
                    lw = winT[:, d * 3 * EPC + et * 128:
                              d * 3 * EPC + (et + 1) * 128]
                    for sch in range(2):
                        nc.tensor.matmul(
                            accs[et * 2 + sch][:], lw,
                            xt[:, sch * QC:(sch + 1) * QC],
                            start=(d == 0), stop=(d == ND - 1))
            for sch in range(2):
                c0 = sch * QC
                nc.scalar.copy(qraw[:, c0:c0 + QC], accs[0 * 2 + sch][:])
                nc.scalar.copy(kraw[:, c0:c0 + QC], accs[1 * 2 + sch][:])
                nc.scalar.copy(vtr[:, c0:c0 + QC], accs[2 * 2 + sch][:])

            def pa_tile():
                return psA.tile([128, QC], F32, tag="acc", bufs=6, name="rp")
            for sch in range(2):
                rope_chunk(sch * QC, pa_tile)
            def pa_vt():
                return psA.tile([128, 128], F32R, tag="acc", bufs=6,
                                name="vp")
            for j in range(8):
                vt_tile(j, pa_vt)

        # ===== Stage B/C interleaved with stage A second half ===========
        with tc.tile_pool(name="psB", bufs=1, space="PSUM") as psB, \
             tc.tile_pool(name="wkb", bufs=3) as wkb:

            def op_tile():
                return psB.tile([128, QC], F32, tag="op", bufs=2, name="op")

            def norm_thunks(qc, pvs):
                thunks = []
                rcps = []
                for hh in range(2):
                    rcp = wkb.tile([65, QC], F32, tag="rcp", bufs=4,
                                   name="rcp")
                    with nc.allow_low_precision(reason="denom recip"):
                        nc.vector.reciprocal(rcp[64:65, :], pvs[hh][64:65, :])
                    rcps.append(rcp)

                def norm(hh):
                    def f():
                        q0 = qc * QC
                        rb = op_tile()
                        nc.tensor.matmul(rb[0:HD, :], onesf[64:65, 0:HD],
                                         rcps[hh][64:65, :],
                                         start=True, stop=True)
                        rbs = wkb.tile([HD, QC], F32, tag="rbs", bufs=2,
                                       name="rbs")
                        nc.scalar.copy(rbs[:], rb[0:HD, :])
                        ctxh = ctx0 if hh == 0 else ctx1
                        nc.vector.tensor_mul(ctxh[:, q0:q0 + QC],
                                             pvs[hh][0:64, :], rbs[:])
                    return f

                def oproj(sti):
                    def f():
                        c0 = (qc * 4 + sti) * 128
                        ob = wkb.tile([128, D], F32, tag="ob", bufs=3,
                                      name="ob")
                        for dc in range(2):
                            op = op_tile()
                            nc.tensor.matmul(op[:], ctx0[:, c0:c0 + 128],
                                             woT0[:, dc * QC:(dc + 1) * QC],
                                             start=True, stop=False)
                            nc.tensor.matmul(op[:], ctx1[:, c0:c0 + 128],
                                             woT1[:, dc * QC:(dc + 1) * QC],
                                             start=False, stop=True)
                            if dc == 0:
                                nc.scalar.copy(ob[:, dc * QC:(dc + 1) * QC],
                                               op[:])
                            else:
                                nc.vector.tensor_copy(
                                    ob[:, dc * QC:(dc + 1) * QC], op[:])
                        nc.sync.dma_start(pout_d[c0:c0 + 128, :], ob[:])
                    return f

                thunks.append(norm(0))
                thunks.append(norm(1))
                for sti in range(4):
                    thunks.append(oproj(sti))
                return thunks

            LAG = 2

            def attention_chunk(qc, deferred):
                q0 = qc * QC
                n_k = 4 * (qc + 1) if causal else NST
                pvs = [psB.tile([65, QC], F32, tag="pv", bufs=4,
                                name=f"pv{hh}") for hh in range(2)]
                window = []

                def emit_pv(pkt, p0, p1, last):
                    js = max(0, pkt - qc * 4) * 128 if causal else 0
                    for hh, pp in ((0, p0), (1, p1)):
                        nc.tensor.matmul(
                            pvs[hh][:, js:QC],
                            vnat[:, pkt * 130 + hh * 65:
                                 pkt * 130 + hh * 65 + 65],
                            pp[:, js:QC], start=(pkt == 0), stop=last)

                for kt in range(n_k):
                    pts = []
                    for hh in range(2):
                        st = psB.tile([128, QC], F32, tag="st", bufs=2,
                                      name="st")
                        nc.tensor.matmul(
                            st[:],
                            krot[hh * 64:(hh + 1) * 64,
                                 kt * 128:(kt + 1) * 128],
                            qrot[hh * 64:(hh + 1) * 64, q0:q0 + QC],
                            start=True, stop=True)
                        pt = wkb.tile([128, QC], F32R, tag="pt", bufs=6,
                                      name="pt")
                        j = kt - qc * 4
                        if causal and j >= 0:
                            nc.scalar.activation(
                                pt[:, j * 128:QC], st[:, j * 128:QC],
                                mybir.ActivationFunctionType.Exp, scale=0.125)
                            nc.vector.tensor_mul(
                                pt[:, j * 128:(j + 1) * 128],
                                pt[:, j * 128:(j + 1) * 128], tri[:])
                        else:
                            nc.scalar.activation(
                                pt[:], st[:],
                                mybir.ActivationFunctionType.Exp, scale=0.125)
                        pts.append(pt)
                    window.append((kt, pts[0], pts[1]))
                    if len(window) > LAG:
                        emit_pv(*window.pop(0), last=False)
                    if deferred and kt >= 3 and kt % 2 == 1:
                        deferred.pop(0)()
                while window:
                    kt_, a_, b_ = window.pop(0)
                    emit_pv(kt_, a_, b_, last=(kt_ == n_k - 1))
                while deferred:
                    deferred.pop(0)()
                return pvs

            # warm-up burst bridging the rope->attention dependency gap
            heat = psB.tile([128, QC], F32, tag="st", bufs=2, name="heat")
            for _ in range(16):
                nc.tensor.matmul(heat[:], winT[:, 0:128], winT[:, 1024:1536],
                                 start=True, stop=True)
            nc.scalar.copy(heatout[:], heat[0:1, 0:1])

            pvs0 = attention_chunk(0, [])
            pvs1 = attention_chunk(1, norm_thunks(0, pvs0))
            d1 = norm_thunks(1, pvs1)

            # ---- Stage A second half, accumulators in the "op" slots ----
            # (xT half-1 DMAs landed during chunks 0/1; thunks for chunk 1
            #  interleave between accumulation groups)
            xts = []
            for d in range(ND):
                xt = wkb.tile([128, QC], F32R, tag="xt2", bufs=8, name="xt2")
                nc.sync.dma_start(
                    xt[:], xT_d[d * 128:(d + 1) * 128, S // 2:S // 2 + QC])
                xts.append(xt)
            for sch in range(2):
                c0 = S // 2 + sch * QC
                if sch == 1:
                    xts = []
                    for d in range(ND):
                        xt = wkb.tile([128, QC], F32R, tag="xt2", bufs=8,
                                      name="xt2")
                        nc.sync.dma_start(
                            xt[:], xT_d[d * 128:(d + 1) * 128, c0:c0 + QC])
                        xts.append(xt)
                for et in range(3):
                    acc = op_tile()
                    for d in range(ND):
                        lw = winT[:, d * 3 * EPC + et * 128:
                                  d * 3 * EPC + (et + 1) * 128]
                        nc.tensor.matmul(acc[:], lw, xts[d][:],
                                         start=(d == 0), stop=(d == ND - 1))
                    dst = (qraw, kraw, vtr)[et]
                    nc.scalar.copy(dst[:, c0:c0 + QC], acc[:])
                    if d1:
                        d1.pop(0)()
            for sch in range(2):
                rope_chunk(S // 2 + sch * QC, op_tile)
                if d1:
                    d1.pop(0)()
            def op_vt():
                return psB.tile([128, 128], F32R, tag="op", bufs=2,
                                name="vp")
            for j in range(8, 16):
                vt_tile(j, op_vt)
            while d1:
                d1.pop(0)()

            pvs2 = attention_chunk(2, [])
            pvs3 = attention_chunk(3, norm_thunks(2, pvs2))
            for t in norm_thunks(3, pvs3):
                t()

    _split_multi_waits(nc)
    return nc


_CONSTS = _host_constants()
_PROGRAMS = {}


def _get_program(causal: bool):
    if causal not in _PROGRAMS:
        _PROGRAMS[causal] = _build_program(causal)
    return _PROGRAMS[causal]


def kernel(x, w_in, w_out, is_causal):
    causal = bool(np.asarray(is_causal).item())
    nc = _get_program(causal)

    x2 = np.asarray(x, dtype=np.float32).reshape(S, D)
    xT = np.ascontiguousarray(x2.T)                       # [D, S]
    w_in = np.asarray(w_in, dtype=np.float32)
    w_out = np.asarray(w_out, dtype=np.float32)

    in_maps = []
    for c in range(NCORES):
        r0 = c * EPC
        wq = w_in[r0:r0 + EPC, :]                          # [128, D]
        wk = w_in[D + r0:D + r0 + EPC, :]
        wv = w_in[2 * D + r0:2 * D + r0 + EPC, :]
        winT = np.ascontiguousarray(
            np.concatenate([wq, wk, wv], axis=0).T)        # [D, 384]
        woT0 = np.ascontiguousarray(w_out[:, r0:r0 + HD].T)        # [64, D]
        woT1 = np.ascontiguousarray(w_out[:, r0 + HD:r0 + EPC].T)  # [64, D]
        in_maps.append({"xT": xT, "winT": winT, "woT0": woT0, "woT1": woT1,
                        "rotT": _CONSTS[2], "tri": _CONSTS[3],
                        "ident": _CONSTS[4]})

    res = run_bass_kernel_spmd(nc, in_maps, list(range(NCORES)))
    out = np.zeros((S, D), dtype=np.float64)
    for c in range(NCORES):
        out += res.results[c]["pout"].astype(np.float64)
    return out.astype(np.float32).reshape(B, S, D)
